# revision 37
# baseline (speedup 1.0000x reference)
"""Trainium2 Bass kernel for sparse causal attention (nn_CausalAttentionKV).

Reference computation (fp32, single device):
    q_all = x @ Wq + bq ; k_all = x @ Wk + bk ; v_all = x @ Wv + bv
    q = gather(q_all, query_idx)        # (B, M, D) selected query rows
    att = softmax(mask(q k^T / sqrt(hd)))   # per-query causal mask t <= qidx[m]
    y = (att v) @ Wo + bo

Shapes: B=4, T=4096, D=2048, n_head=16, hd=128, M=512.

Sharding (8 cores): core = 2*b + g  handles batch b and head-group g
(8 heads = 1024 feature cols).  Q/K/V projections are column-parallel,
out-proj is row-parallel; the two partial outputs per batch are summed
on the host.  All matmul inputs are bf16 (fp32 PSUM accumulation).

Scheduling notes (measured 630 us HW vs 663 us for the previous
version; ~566 us is the bf16 PE streaming floor for this split):
- dma_start costs ~0.6us of sync-engine descriptor generation each, so
  every operand is one SBUF supertile filled by a handful of large
  fully-contiguous transfers (host pre-packs [partition][contig]
  layouts), issue-ordered to match consumption.  Reusing-tile reloads
  (wq high half) must be EMITTED after the full consumer group
  including its PSUM-draining activations, or results corrupt.
- Phase A runs d-chunk-outer: Q, then K/V per 512-t chunk, rotating
  4-bank groups through one 8-buffer PSUM pool; the PE starts ~13us in
  (7us framework prologue + first 512KB) and never drains at the
  A-Q -> A-KV seam.
- The causal mask is applied MULTIPLICATIVELY (0/1) to e = exp(s) on
  SBUF after the activation, not additively on PSUM before it: the
  PSUM score tile is freed by the exp itself, shortening the
  PE -> ACT recycle chain that throttled the attention pipeline.
- Phase B: both softmax row-sum accumulators share one PSUM bank
  (partitions 0/32), the norm broadcast has its own bank, K^T is
  prefetched two heads ahead (3 buffers), wide/narrow chunk pairs are
  interleaved so the big exps don't cluster, the PV/row-sum drain lags
  the score/exp stream by TWO batches (dependency-free PE filler), and
  all column windows are 4-aligned for the DVE/ACT wide modes.
- Phase C writes bf16 partial outputs through one staging tile per
  128-row block (4 dma_starts total); the host sums the two partials
  per batch in fp32.
"""

import sys
import types
from contextlib import ExitStack

import numpy as np
import ml_dtypes

import concourse.bass as bass
import concourse.tile as tile
import concourse.mybir as mybir
from concourse import bacc
from concourse.bass_utils import run_bass_kernel_spmd

BF16 = mybir.dt.bfloat16
F32 = mybir.dt.float32
NPBF = ml_dtypes.bfloat16

B, T, D = 4, 4096, 2048
NH, HD, M = 16, 128, 512
NHG = 8            # heads per core (group)
DG = NHG * HD      # 1024 feature cols per core
NT = T // 128      # 32 t-chunks
ND = D // 128      # 16 d-chunks
KTS = 512


def _install_ntff_hook():
    """Register the axon NTFF profiling hook if the image's antenv lacks it."""
    try:
        from antenv.axon_hooks import get_axon_ntff_profile_hook  # noqa: F401
        return
    except ImportError:
        pass
    try:
        import antenv
        from trn_agent_boot.trn_boot import _ntff_profile_via_ctypes

        mod = types.ModuleType("antenv.axon_hooks")
        hook = [None]
        mod.set_axon_ntff_profile_hook = lambda h: hook.__setitem__(0, h)
        mod.get_axon_ntff_profile_hook = lambda: hook[0]
        sys.modules["antenv.axon_hooks"] = mod
        antenv.axon_hooks = mod
        mod.set_axon_ntff_profile_hook(
            _ntff_profile_via_ctypes("/opt/axon/libaxon_pjrt.so")
        )
    except Exception:
        pass


def build_program(flo, fhi):
    """Build the per-core Bass program.

    flo[i]: first m column with any allowed key in t-chunk i (cols below
            are fully masked there -> never computed).
    fhi[i]: first m column fully allowed in t-chunk i (cols beyond need
            no mask).
    Both are unions over the 4 batches so one program serves all cores.
    """
    nc = bacc.Bacc("TRN2", target_bir_lowering=False, debug=False)

    # host-packed layouts: per-partition fully contiguous runs
    xtp_d = nc.dram_tensor("xtp", [T // KTS, 128, ND, KTS], BF16, kind="ExternalInput")
    xqp_d = nc.dram_tensor("xqp", [128, ND, M], BF16, kind="ExternalInput")
    wkp_d = nc.dram_tensor("wkp", [128, ND, DG], BF16, kind="ExternalInput")
    wvp_d = nc.dram_tensor("wvp", [128, ND, DG], BF16, kind="ExternalInput")
    wqp_d = nc.dram_tensor("wqp", [128, ND, DG], BF16, kind="ExternalInput")
    wop_d = nc.dram_tensor("wop", [128, NHG, D], BF16, kind="ExternalInput")
    bks = nc.dram_tensor("bks", [128, NHG], F32, kind="ExternalInput")
    bqs = nc.dram_tensor("bqs", [128, NHG], F32, kind="ExternalInput")
    y = nc.dram_tensor("y", [M // 128, 128, D], BF16, kind="ExternalOutput")

    # 0/1 mask super-tiles: 4 t-chunks each, shared col-window.
    # mlo is 4-col aligned so every AP base lands on 8-byte boundaries
    # (keeps DVE/ACT in their wide modes).
    mlo = [min(flo[4 * g: 4 * g + 4]) & ~3 for g in range(NT // 4)]
    mhi = [max(fhi[4 * g: 4 * g + 4]) for g in range(NT // 4)]
    mask_d = {}
    for g in range(NT // 4):
        if mlo[g] < M and mhi[g] > mlo[g]:
            mask_d[g] = nc.dram_tensor(
                f"mask{g}", [128, 4, mhi[g] - mlo[g]], BF16, kind="ExternalInput"
            )

    with ExitStack() as ctx:
        tc = ctx.enter_context(tile.TileContext(nc))

        # ---- persistent tiles --------------------------------------
        persist = ctx.enter_context(tc.tile_pool(name="persist", bufs=1))
        v_t = [persist.tile([128, DG], BF16, name=f"v{i}", tag=f"v{i}") for i in range(NT)]
        qt_t = [persist.tile([128, M], BF16, name=f"qt{j}", tag=f"qt{j}") for j in range(NHG)]
        # ot overlays qt: head j's normalized output overwrites its query
        # tile after the last score matmul read it (WAR dep via Tile)
        ot_t = qt_t
        bias_k = persist.tile([128, NHG], F32, name="bias_k", tag="bias_k")
        bias_q = persist.tile([128, NHG], F32, name="bias_q", tag="bias_q")
        zbias = persist.tile([128, 1], F32, name="zbias", tag="zbias")
        ones_c = persist.tile([128, 1], BF16, name="ones_c", tag="ones_c")
        kt0_sb = persist.tile([128, T], BF16, name="kt0_sb", tag="kt0_sb")
        mask_t = {
            g: persist.tile(
                [128, 4, mhi[g] - mlo[g]], BF16, name=f"mask{g}", tag=f"mask{g}"
            )
            for g in mask_d
        }
        dram = ctx.enter_context(tc.tile_pool(name="dram", bufs=1, space="DRAM"))
        ktd = dram.tile([NHG, 128, T], BF16, name="ktd")
        ktd_r = ktd.rearrange("j p t -> p j t")

        nc.vector.memset(zbias[:], 0.0)
        nc.vector.memset(ones_c[:], 1.0)
        # dummy broadcast so GPSIMD's LOAD_LIB (~7us) happens during the
        # startup DMA window, not at the first phase-B norm
        gpwarm = persist.tile([128, 1], F32, name="gpwarm", tag="gpwarm")
        nc.gpsimd.partition_broadcast(gpwarm[:], zbias[0:1, :])

        inv_s = 1.0 / float(np.sqrt(HD))

        # ---- phase A: projections, d-chunk-outer -------------------
        with (
            nc.named_scope("phase_A"),
            tc.tile_pool(name="wkp", bufs=1) as wkp,
            tc.tile_pool(name="xtp", bufs=2) as xtp,
            tc.tile_pool(name="kst", bufs=2) as kstp,
            tc.tile_pool(name="pa", bufs=8, space="PSUM") as pap,
        ):
            wk_sb = wkp.tile([128, ND, DG], BF16, name="wk", tag="wk")
            xt_sb0 = xtp.tile([128, ND, KTS], BF16, name="xt", tag="xt")

            # A-Q: Qt[j] = ((xq @ wq_j + bq_j)/sqrt(hd))^T, d-outer.
            # wq loads one 4-head column half at a time (the jg=1 half
            # reuses the tile once jg=0's matmuls retire).
            with (
                nc.named_scope("phase_AQ"),
                tc.tile_pool(name="wqp", bufs=1) as wqp,
                tc.tile_pool(name="xqp", bufs=1) as xqp,
            ):
                xq_sb = xqp.tile([128, ND, M], BF16, name="xq", tag="xq")
                wq_sb = wqp.tile([128, ND, 512], BF16, name="wq", tag="wq")
                # issue order = consumption order; first chunks split finest
                # so the PE starts on 256KB, and the wq stream goes through
                # the scalar engine's DGE queue so descriptor generation for
                # the two streams runs in parallel at startup
                nc.sync.dma_start(wq_sb[:, 0:1, :], wqp_d[:, 0:1, 0:512])
                nc.scalar.dma_start(xq_sb[:, 0:1, :], xqp_d[:, 0:1, :])
                nc.sync.dma_start(wq_sb[:, 1:2, :], wqp_d[:, 1:2, 0:512])
                nc.scalar.dma_start(xq_sb[:, 1:2, :], xqp_d[:, 1:2, :])
                nc.sync.dma_start(xq_sb[:, 2:4, :], xqp_d[:, 2:4, :])
                nc.scalar.dma_start(wq_sb[:, 2:4, :], wqp_d[:, 2:4, 0:512])
                nc.sync.dma_start(bias_q[:], bqs[:])
                nc.sync.dma_start(bias_k[:], bks[:])
                for g in range(1, 4):
                    nc.sync.dma_start(
                        xq_sb[:, 4 * g: 4 * g + 4, :], xqp_d[:, 4 * g: 4 * g + 4, :]
                    )
                    nc.scalar.dma_start(
                        wq_sb[:, 4 * g: 4 * g + 4, :],
                        wqp_d[:, 4 * g: 4 * g + 4, 0:512],
                    )
                pq = [pap.tile([128, M], F32, name="pq", tag="pa") for _ in range(NHG)]
                for jg in range(2):
                    for d in range(ND):
                        for jj in range(4):
                            nc.tensor.matmul(
                                pq[4 * jg + jj][:],
                                wq_sb[:, d, jj * 128: (jj + 1) * 128],
                                xq_sb[:, d, :],
                                start=(d == 0),
                                stop=(d == ND - 1),
                                skip_group_check=True,
                            )
                    for jj in range(4):
                        j = 4 * jg + jj
                        nc.scalar.activation(
                            qt_t[j][:],
                            pq[j][:],
                            mybir.ActivationFunctionType.Identity,
                            scale=inv_s,
                            bias=bias_q[:, j: j + 1],
                        )
                    if jg == 0:
                        # wq high half reloads the same tile: emitted after
                        # the jg=0 matmuls (WAR clears group by group) but
                        # ahead of wk/xt0 in the DMA queues
                        for g in range(4):
                            nc.sync.dma_start(
                                wq_sb[:, 4 * g: 4 * g + 4, :],
                                wqp_d[:, 4 * g: 4 * g + 4, 512:DG],
                            )
                        for g in range(4):
                            nc.sync.dma_start(
                                wk_sb[:, 4 * g: 4 * g + 4, 0:512],
                                wkp_d[:, 4 * g: 4 * g + 4, 0:512],
                            )
                            nc.sync.dma_start(
                                xt_sb0[:, 4 * g: 4 * g + 4, :],
                                xtp_d[7, :, 4 * g: 4 * g + 4, :],
                            )
                        for g in range(4):
                            nc.sync.dma_start(
                                wk_sb[:, 4 * g: 4 * g + 4, 512:DG],
                                wkp_d[:, 4 * g: 4 * g + 4, 512:DG],
                            )

            # A-KV: one pass over x computing Kt and V, d-outer groups.
            # wv lands in the space freed by wq/xq during K of ts=0.
            with (
                nc.named_scope("phase_AKV"),
                tc.tile_pool(name="wvp", bufs=1) as wvp,
            ):
                wv_sb = wvp.tile([128, ND, DG], BF16, name="wv", tag="wv")
                for g in range(4):
                    nc.sync.dma_start(
                        wv_sb[:, 4 * g: 4 * g + 4, :], wvp_d[:, 4 * g: 4 * g + 4, :]
                    )
                for g in mask_d:
                    nc.sync.dma_start(mask_t[g][:], mask_d[g][:])
                # ts=7 first: head 0's first lagged drains in phase B read
                # v_t[28:32], so those tiles must not be A's last DVE writes
                for tsi, ts in enumerate([7] + list(range(7))):
                    if tsi == 0:
                        xt_sb = xt_sb0
                    else:
                        xt_sb = xtp.tile([128, ND, KTS], BF16, name="xt", tag="xt")
                        nc.sync.dma_start(xt_sb[:], xtp_d[ts])
                    # K^T: two groups of 4 heads, 4 banks each
                    for jg in range(2):
                        pk = [pap.tile([128, KTS], F32, name="pk", tag="pa") for _ in range(4)]
                        for d in range(ND):
                            for jj in range(4):
                                j = 4 * jg + jj
                                nc.tensor.matmul(
                                    pk[jj][:],
                                    wk_sb[:, d, j * 128: (j + 1) * 128],
                                    xt_sb[:, d, :],
                                    start=(d == 0),
                                    stop=(d == ND - 1),
                                    skip_group_check=True,
                                )
                        ks = kstp.tile([128, 4, KTS], BF16, name="ks", tag="ks")
                        for jj in range(4):
                            j = 4 * jg + jj
                            nc.scalar.activation(
                                kt0_sb[:, ts * KTS: (ts + 1) * KTS] if j == 0
                                else ks[:, jj, :],
                                pk[jj][:],
                                mybir.ActivationFunctionType.Identity,
                                bias=bias_k[:, j: j + 1],
                            )
                        nc.sync.dma_start(
                            ktd_r[:, 4 * jg: 4 * jg + 4, ts * KTS: (ts + 1) * KTS],
                            ks[:],
                        )
                    # V: two f-groups of 4 t-subchunks, 4 banks each
                    for f in range(2):
                        pv = [pap.tile([128, 512], F32, name="pv", tag="pa") for _ in range(4)]
                        for d in range(ND):
                            for u in range(4):
                                nc.tensor.matmul(
                                    pv[u][:],
                                    xt_sb[:, d, u * 128: (u + 1) * 128],
                                    wv_sb[:, d, f * 512: (f + 1) * 512],
                                    start=(d == 0),
                                    stop=(d == ND - 1),
                                    skip_group_check=True,
                                )
                        for u in range(4):
                            nc.vector.tensor_copy(
                                v_t[ts * 4 + u][:, f * 512: (f + 1) * 512], pv[u][:]
                            )

        # ---- phase B prefetch: out-proj weights --------------------
        # (issued lazily inside phase B, 1MB per head boundary, so the 4MB
        # doesn't hog HBM bandwidth while the kth prefetches race B's start)
        wop = ctx.enter_context(tc.tile_pool(name="wop", bufs=1))
        wo_sb = wop.tile([128, NHG, D], BF16, name="wo", tag="wo")

        # ---- phase B: attention per head -----------------------------
        # Wide chunks (lo < 256) go through 2-bank pair supertiles with a
        # 2-buffer pool (distance-4 recycle hides the big exps); narrow
        # chunks (lo >= 256) are batched as QUADS into one single-buffered
        # 2-bank supertile: ONE exp + ONE mask-mul per 4 chunks, cutting the
        # 352-cycle fixed ACTIVATE cost (12 exps/head instead of 16).
        # A batch is [wide, narrow-quad, wide]: widest wide pairs with the
        # narrowest quad, so per-batch ACT load is even and the two PSUM
        # pools ping-pong.
        wide_pairs, narrow_quads = [], []
        for g in range(NT // 4):
            q = [i for i in range(4 * g, 4 * g + 4) if flo[i] < M]
            if not q:
                continue
            if (min(flo[i] for i in q) & ~3) >= 256:
                narrow_quads.append(q)
            else:
                wide_pairs += [q[k: k + 2] for k in (0, 2) if q[k: k + 2]]

        def _mk_batches(narrow_first):
            ws = list(wide_pairs)            # widest first
            ns = list(narrow_quads)[::-1]    # narrowest first
            batches = []
            while ws or ns:
                b = []
                if ws:
                    b.append(("w", ws.pop(0)))
                if ns:
                    b.append(("n", ns.pop(0)))
                if ws:
                    b.append(("w", ws.pop(0)))
                if narrow_first and len(b) > 1 and b[0][0] == "w" and b[1][0] == "n":
                    b[0], b[1] = b[1], b[0]
                batches.append(b)
            return batches

        batches_by_head = [_mk_batches(j == 0) for j in range(NHG)]
        with (
            nc.named_scope("phase_B"),
            tc.tile_pool(name="kth", bufs=3) as kthp,
            tc.tile_pool(name="ps", bufs=2, space="PSUM") as psp,
            tc.tile_pool(name="psn", bufs=1, space="PSUM") as psnp,
            tc.tile_pool(name="po", bufs=1, space="PSUM") as pop,
            tc.tile_pool(name="pl", bufs=1, space="PSUM") as plp,
            tc.tile_pool(name="esb", bufs=5) as esb,
            tc.tile_pool(name="lsb", bufs=1) as lsb,
        ):
            po_q, pl_q = {}, {}
            # both row-sum accumulators share one bank (partitions 0/32)
            pl_t = plp.tile([64, M], F32, name="pl", tag="pl")

            def emit_norm(j):
                """Normalize head j: ot[j] = po[j] / l[j] (off PE critical path).

                The reciprocal row is broadcast across partitions on GPSIMD
                (idle engine) instead of a PE matmul + ACT copy, freeing a
                PSUM bank for the score pipeline.
                """
                po, pl = po_q.pop(j), pl_q.pop(j)
                l_sb = lsb.tile([1, M], F32, name="l", tag="l")
                linv = lsb.tile([1, M], F32, name="linv", tag="linv")
                nc.vector.tensor_copy(l_sb[:], pl)
                nc.vector.reciprocal_approx_fast(linv[:], l_sb[:])
                lb_sb = lsb.tile([128, M], F32, name="lb", tag="lb")
                nc.gpsimd.partition_broadcast(lb_sb[:], linv[:])
                nc.vector.tensor_mul(ot_t[j][:], po[:], lb_sb[:])

            kth = {0: kt0_sb}
            issued = {0}

            def ensure_kth(jn):
                if jn < NHG and jn not in issued:
                    kth[jn] = kthp.tile([128, T], BF16, name="kth", tag="kth")
                    nc.sync.dma_start(kth[jn][:], ktd[jn])
                    issued.add(jn)

            state = {}  # j -> [po_start_pending, l_start_pending]

            def drain(pj, cur, rsl, last_b):
                """Emit the lagged PV + row-sum matmuls for head pj's batch."""
                if pj not in po_q:
                    # lazy: with norm emitted before the first drain of the
                    # next head, only one PV accumulator bank is live
                    po_q[pj] = pop.tile([128, M], F32, name="po", tag="po")
                st = state[pj]
                for k, (pair, e, uoff, lo) in enumerate(cur):
                    for u, i in enumerate(pair):
                        nc.tensor.matmul(
                            po_q[pj][:, lo:M],
                            v_t[i][:, pj * 128: (pj + 1) * 128],
                            e[:, uoff + u, lo:M],
                            start=st[0],
                            stop=(last_b and k == len(cur) - 1 and u == len(pair) - 1),
                            skip_group_check=True,
                        )
                        st[0] = False
                for k, (es2, uoff, lo) in enumerate(rsl):
                    rs = es2[:, lo:M] if uoff is None else es2[:, uoff, lo:M]
                    nc.tensor.matmul(
                        pl_q[pj][:, lo:M], ones_c[:], rs,
                        start=st[1], stop=(last_b and k == len(rsl) - 1),
                        skip_group_check=True,
                    )
                    st[1] = False

            # flat (head, batch) pipeline: PV/l lag the S/exp stream by TWO
            # batches ACROSS head boundaries, so the PE always has
            # dependency-free drain work to absorb exp latency
            pendq = []  # [(j, cur, is_last_batch_of_head)]
            for j in range(NHG):
                ensure_kth(j + 1)
                if j < 4:
                    # out-proj weights trickle in 1MB/head behind the kth
                    # prefetches; only needed at phase C
                    nc.sync.dma_start(
                        wo_sb[:, 2 * j: 2 * j + 2, :], wop_d[:, 2 * j: 2 * j + 2, :]
                    )
                pl_q[j] = pl_t[32 * (j % 2): 32 * (j % 2) + 1, :]
                state[j] = [True, True]
                batches = batches_by_head[j]
                for bi, batch in enumerate(batches):
                    # drain first: with the lag-3 window this finishes head
                    # j-1's PV/l accumulation by bi==2, a full batch before
                    # its po bank partner is written again
                    if len(pendq) >= 3:
                        drain(*pendq.pop(0))
                    if bi == 2 and j > 0 and (j - 1) in po_q:
                        emit_norm(j - 1)
                    cur = []   # PV entries: (pair, e, uoff, lo_pv)
                    rsl = []   # row-sum entries: (tile, uoff|None, lo)
                    qacc = {}  # g -> [esum...] for the quad-combine
                    for kind, grp in batch:
                        g = grp[0] // 4
                        ng = len(grp)
                        # all groups use their QUAD's lo as the column base
                        # so a quad's two pair-sums can be combined into ONE
                        # row-sum matmul (widens wide-pair-b scores by ~32
                        # cols; the mask zeroes the extra band)
                        lo_g = mlo[g]
                        if kind == "n":
                            pst = psnp.tile([128, 4, 256], F32, name="pstn", tag="psn")
                            off = 256
                        else:
                            pst = psp.tile([128, 2, M], F32, name="pst", tag="ps")
                            off = 0
                        for u, i in enumerate(grp):
                            nc.tensor.matmul(
                                pst[:, u, lo_g - off: M - off],
                                kth[j][:, i * 128: (i + 1) * 128],
                                qt_t[j][:, lo_g:M],
                                start=True,
                                stop=True,
                                skip_group_check=True,
                            )
                        # ONE exp straight off PSUM for the whole group
                        # (frees the score tile), then ONE multiplicative 0/1
                        # causal mask over the partial band
                        if kind == "n":
                            e = esb.tile([128, 4, M], BF16, name="e4", tag="e4", bufs=5)
                        else:
                            e = esb.tile([128, 2, M], BF16, name="e2", tag="e2", bufs=8)
                        nc.scalar.activation(
                            e[:, :ng, lo_g:M],
                            pst[:, :ng, lo_g - off: M - off],
                            mybir.ActivationFunctionType.Exp,
                            bias=zbias[:],
                        )
                        fhi_max = max(fhi[i] for i in grp)
                        um = grp[0] % 4
                        if lo_g < fhi_max:
                            nc.vector.tensor_mul(
                                e[:, :ng, lo_g:fhi_max],
                                e[:, :ng, lo_g:fhi_max],
                                mask_t[g][:, um: um + ng, lo_g - mlo[g]: fhi_max - mlo[g]],
                            )
                        for uo in range(0, ng, 2):
                            pair = grp[uo: uo + 2]
                            lo_pv = min(flo[i] for i in pair) & ~3
                            if len(pair) == 2:
                                # pair-sum on DVE over the quad's full range
                                esum = esb.tile(
                                    [128, M], BF16, name="esum", tag="esum", bufs=14
                                )
                                nc.vector.tensor_add(
                                    esum[:, lo_g:M],
                                    e[:, uo, lo_g:M],
                                    e[:, uo + 1, lo_g:M],
                                )
                                qacc.setdefault(g, []).append(esum)
                            else:
                                rsl.append((e, uo, lo_g))
                            cur.append((pair, e, uo, lo_pv))
                    for g, lst in qacc.items():
                        if len(lst) == 2:
                            # quad-combine on GPSIMD (idle engine): ONE
                            # row-sum matmul per 4 chunks instead of per pair
                            es2 = esb.tile([128, M], BF16, name="es2", tag="es2", bufs=8)
                            nc.gpsimd.tensor_add(
                                es2[:, mlo[g]:M], lst[0][:, mlo[g]:M], lst[1][:, mlo[g]:M]
                            )
                            rsl.append((es2, None, mlo[g]))
                        else:
                            rsl.append((lst[0], None, mlo[g]))
                    pendq.append((j, cur, rsl, bi == len(batches) - 1))
                    if bi == 1:
                        ensure_kth(j + 2)
            for p in pendq:
                drain(*p)
            emit_norm(NHG - 1)

        # ---- phase C: y = O @ wo  (row-parallel partial, bf16 out) -
        with (
            nc.named_scope("phase_C"),
            tc.tile_pool(name="py", bufs=2, space="PSUM") as pyp,
            tc.tile_pool(name="ysb", bufs=2) as ysb,
        ):
            # output DMAs alternate across the two DGE queues so the 2MB of
            # y doesn't serialize on one ring at the very end; the last
            # block ships in two halves to shorten the post-matmul tail
            for mb in range(M // 128):
                ys = ysb.tile([128, D], BF16, name="ys", tag="ys")
                last = mb == M // 128 - 1
                dma_eng = nc.sync if mb % 2 == 0 else nc.scalar
                for fp in range(D // 1024):
                    py = [
                        pyp.tile([128, 512], F32, name="py", tag=f"py{2 * (fp % 2) + h}")
                        for h in range(2)
                    ]
                    for j in range(NHG):
                        for h in range(2):
                            fo = 2 * fp + h
                            nc.tensor.matmul(
                                py[h][:],
                                ot_t[j][:, mb * 128: (mb + 1) * 128],
                                wo_sb[:, j, fo * 512: (fo + 1) * 512],
                                start=(j == 0),
                                stop=(j == NHG - 1),
                                skip_group_check=True,
                            )
                    for h in range(2):
                        fo = 2 * fp + h
                        nc.scalar.copy(ys[:, fo * 512: (fo + 1) * 512], py[h][:])
                        if last:
                            dma_eng = nc.sync if fo % 2 == 0 else nc.scalar
                            dma_eng.dma_start(
                                y[mb][:, fo * 512: (fo + 1) * 512],
                                ys[:, fo * 512: (fo + 1) * 512],
                            )
                if not last:
                    dma_eng.dma_start(y[mb], ys[:])

    nc.compile()
    return nc


_cache = {}


def _get_program(flo, fhi):
    key = (tuple(flo), tuple(fhi))
    if key not in _cache:
        _cache[key] = build_program(list(flo), list(fhi))
    return _cache[key]


def _packc(a):
    """[C*128, N] row-major -> [128][C][N]: per-partition contiguous runs."""
    c = a.shape[0] // 128
    return np.ascontiguousarray(a.reshape(c, 128, a.shape[1]).transpose(1, 0, 2))


def _prep(inputs):
    x = np.asarray(inputs["x"], dtype=np.float32)
    qidx = np.asarray(inputs["query_idx"]).astype(np.int64)
    Wq = np.asarray(inputs["Wq"], dtype=np.float32)
    Wk = np.asarray(inputs["Wk"], dtype=np.float32)
    Wv = np.asarray(inputs["Wv"], dtype=np.float32)
    Wo = np.asarray(inputs["Wo"], dtype=np.float32)
    bq = np.asarray(inputs["bq"], dtype=np.float32)
    bk = np.asarray(inputs["bk"], dtype=np.float32)
    bv = np.asarray(inputs["bv"], dtype=np.float32)
    bo = np.asarray(inputs["bo"], dtype=np.float32)

    # Per-t-chunk skip bounds, union over batches.  flo[i] = first m that
    # attends into chunk i (everything below is fully masked there);
    # fhi[i] = one past the last m only partially covered by chunk i.
    # Computed positionally so they are correct even for unsorted
    # query_idx (just less effective at skipping).
    flo = [M] * NT
    fhi = [0] * NT
    for b in range(B):
        for i in range(NT):
            allowed = qidx[b] >= 128 * i          # chunk i not fully masked
            partial = qidx[b] < 128 * (i + 1)     # chunk i not fully allowed
            lo_b = int(np.argmax(allowed)) if allowed.any() else M
            hi_b = M - int(np.argmax(partial[::-1])) if partial.any() else 0
            flo[i] = min(flo[i], lo_b)
            fhi[i] = max(fhi[i], hi_b)
    mlo = [min(flo[4 * g: 4 * g + 4]) & ~3 for g in range(NT // 4)]
    mhi = [max(fhi[4 * g: 4 * g + 4]) for g in range(NT // 4)]

    in_maps = []
    tgrid = np.arange(T)[:, None]
    for core in range(8):
        b, g = divmod(core, 2)
        sl = slice(g * DG, (g + 1) * DG)
        xb = x[b]
        xT = xb.T.astype(NPBF)                                # [D, T]
        # [ts][128][ND][KTS]: per-partition 16KB contiguous runs
        xtp = np.ascontiguousarray(
            xT.reshape(ND, 128, T // KTS, KTS).transpose(2, 1, 0, 3)
        )
        # 0/1 multiplicative causal mask
        mask = (tgrid <= qidx[b][None, :]).astype(NPBF)
        mask4 = mask.reshape(NT, 128, M)
        im = {
            "xtp": xtp,
            "xqp": _packc(xb[qidx[b]].T.astype(NPBF)),
            "wkp": _packc(Wk[:, sl].astype(NPBF)),
            "wvp": _packc(Wv[:, sl].astype(NPBF)),
            "wqp": _packc(Wq[:, sl].astype(NPBF)),
            "wop": _packc(Wo[sl, :].astype(NPBF)),
            "bks": np.ascontiguousarray(bk[sl].reshape(NHG, 128).T),
            "bqs": np.ascontiguousarray(
                (bq[sl] / np.sqrt(HD)).reshape(NHG, 128).T.astype(np.float32)
            ),
        }
        for g4 in range(NT // 4):
            if mlo[g4] < M and mhi[g4] > mlo[g4]:
                im[f"mask{g4}"] = np.ascontiguousarray(
                    mask4[4 * g4: 4 * g4 + 4, :, mlo[g4]: mhi[g4]].transpose(1, 0, 2)
                )
        in_maps.append(im)

    const = (bv.astype(np.float64) @ Wo.astype(np.float64) + bo).astype(np.float32)
    return flo, fhi, in_maps, const


def run(inputs, trace=False, trace_kwargs=None):
    _install_ntff_hook()
    flo, fhi, in_maps, const = _prep(inputs)
    nc = _get_program(flo, fhi)
    res = run_bass_kernel_spmd(
        nc, in_maps, list(range(8)), trace=trace, **(trace_kwargs or {})
    )
    out = np.zeros((B, M, D), dtype=np.float32)
    for b in range(B):
        out[b] = (
            res.results[2 * b]["y"].reshape(M, D).astype(np.float32)
            + res.results[2 * b + 1]["y"].reshape(M, D).astype(np.float32)
            + const
        )
    return out, res


def kernel(**inputs) -> np.ndarray:
    out, _ = run(inputs, trace=False)
    return out



# revision 39
# speedup vs baseline: 1.1971x; 1.1971x over previous
"""Trainium2 Bass kernel for sparse causal attention (nn_CausalAttentionKV).

Reference computation (fp32, single device):
    q_all = x @ Wq + bq ; k_all = x @ Wk + bk ; v_all = x @ Wv + bv
    q = gather(q_all, query_idx)        # (B, M, D) selected query rows
    att = softmax(mask(q k^T / sqrt(hd)))   # per-query causal mask t <= qidx[m]
    y = (att v) @ Wo + bo

Shapes: B=4, T=4096, D=2048, n_head=16, hd=128, M=512.

Sharding (8 cores): core = 2*b + g  handles batch b and head-group g
(8 heads = 1024 feature cols).  Q/K/V projections are column-parallel,
out-proj is row-parallel; the two partial outputs per batch are summed
on the host.  All matmul inputs are bf16 (fp32 PSUM accumulation).

Scheduling notes (measured 630 us HW vs 663 us for the previous
version; ~566 us is the bf16 PE streaming floor for this split):
- dma_start costs ~0.6us of sync-engine descriptor generation each, so
  every operand is one SBUF supertile filled by a handful of large
  fully-contiguous transfers (host pre-packs [partition][contig]
  layouts), issue-ordered to match consumption.  Reusing-tile reloads
  (wq high half) must be EMITTED after the full consumer group
  including its PSUM-draining activations, or results corrupt.
- Phase A runs d-chunk-outer: Q, then K/V per 512-t chunk, rotating
  4-bank groups through one 8-buffer PSUM pool; the PE starts ~13us in
  (7us framework prologue + first 512KB) and never drains at the
  A-Q -> A-KV seam.
- The causal mask is applied MULTIPLICATIVELY (0/1) to e = exp(s) on
  SBUF after the activation, not additively on PSUM before it: the
  PSUM score tile is freed by the exp itself, shortening the
  PE -> ACT recycle chain that throttled the attention pipeline.
- Phase B: both softmax row-sum accumulators share one PSUM bank
  (partitions 0/32), the norm broadcast has its own bank, K^T is
  prefetched two heads ahead (3 buffers), wide/narrow chunk pairs are
  interleaved so the big exps don't cluster, the PV/row-sum drain lags
  the score/exp stream by TWO batches (dependency-free PE filler), and
  all column windows are 4-aligned for the DVE/ACT wide modes.
- Phase C writes bf16 partial outputs through one staging tile per
  128-row block (4 dma_starts total); the host sums the two partials
  per batch in fp32.
"""

import sys
import types
from contextlib import ExitStack

import numpy as np
import ml_dtypes

import concourse.bass as bass
import concourse.tile as tile
import concourse.mybir as mybir
from concourse import bacc
from concourse.bass_utils import run_bass_kernel_spmd

BF16 = mybir.dt.bfloat16
F32 = mybir.dt.float32
NPBF = ml_dtypes.bfloat16

B, T, D = 4, 4096, 2048
NH, HD, M = 16, 128, 512
NHG = 8            # heads per core (group)
DG = NHG * HD      # 1024 feature cols per core
NT = T // 128      # 32 t-chunks
ND = D // 128      # 16 d-chunks
KTS = 512


def _install_ntff_hook():
    """Register the axon NTFF profiling hook if the image's antenv lacks it."""
    try:
        from antenv.axon_hooks import get_axon_ntff_profile_hook  # noqa: F401
        return
    except ImportError:
        pass
    try:
        import antenv
        from trn_agent_boot.trn_boot import _ntff_profile_via_ctypes

        mod = types.ModuleType("antenv.axon_hooks")
        hook = [None]
        mod.set_axon_ntff_profile_hook = lambda h: hook.__setitem__(0, h)
        mod.get_axon_ntff_profile_hook = lambda: hook[0]
        sys.modules["antenv.axon_hooks"] = mod
        antenv.axon_hooks = mod
        mod.set_axon_ntff_profile_hook(
            _ntff_profile_via_ctypes("/opt/axon/libaxon_pjrt.so")
        )
    except Exception:
        pass


def build_program(flo, fhi):
    """Build the per-core Bass program.

    flo[i]: first m column with any allowed key in t-chunk i (cols below
            are fully masked there -> never computed).
    fhi[i]: first m column fully allowed in t-chunk i (cols beyond need
            no mask).
    Both are unions over the 4 batches so one program serves all cores.
    """
    nc = bacc.Bacc("TRN2", target_bir_lowering=False, debug=False)

    # host-packed layouts: per-partition fully contiguous runs
    xtp_d = nc.dram_tensor("xtp", [T // KTS, 128, ND, KTS], BF16, kind="ExternalInput")
    xqp_d = nc.dram_tensor("xqp", [128, ND, M], BF16, kind="ExternalInput")
    wkp_d = nc.dram_tensor("wkp", [128, ND, DG], BF16, kind="ExternalInput")
    wvp_d = nc.dram_tensor("wvp", [128, ND, DG], BF16, kind="ExternalInput")
    wqp_d = nc.dram_tensor("wqp", [128, ND, DG], BF16, kind="ExternalInput")
    wop_d = nc.dram_tensor("wop", [128, NHG, D], BF16, kind="ExternalInput")
    bks = nc.dram_tensor("bks", [128, NHG], F32, kind="ExternalInput")
    bqs = nc.dram_tensor("bqs", [128, NHG], F32, kind="ExternalInput")
    y = nc.dram_tensor("y", [M // 128, 128, D], BF16, kind="ExternalOutput")

    # 0/1 mask super-tiles: 4 t-chunks each, shared col-window.
    # mlo is 4-col aligned so every AP base lands on 8-byte boundaries
    # (keeps DVE/ACT in their wide modes).
    mlo = [min(flo[4 * g: 4 * g + 4]) & ~3 for g in range(NT // 4)]
    mhi = [max(fhi[4 * g: 4 * g + 4]) for g in range(NT // 4)]
    mask_d = {}
    for g in range(NT // 4):
        if mlo[g] < M and mhi[g] > mlo[g]:
            mask_d[g] = nc.dram_tensor(
                f"mask{g}", [128, 4, mhi[g] - mlo[g]], BF16, kind="ExternalInput"
            )

    with ExitStack() as ctx:
        tc = ctx.enter_context(tile.TileContext(nc))

        # ---- persistent tiles --------------------------------------
        persist = ctx.enter_context(tc.tile_pool(name="persist", bufs=1))
        v_t = [persist.tile([128, DG], BF16, name=f"v{i}", tag=f"v{i}") for i in range(NT)]
        qt_t = [persist.tile([128, M], BF16, name=f"qt{j}", tag=f"qt{j}") for j in range(NHG)]
        # ot overlays qt: head j's normalized output overwrites its query
        # tile after the last score matmul read it (WAR dep via Tile)
        ot_t = qt_t
        bias_k = persist.tile([128, NHG], F32, name="bias_k", tag="bias_k")
        bias_q = persist.tile([128, NHG], F32, name="bias_q", tag="bias_q")
        zbias = persist.tile([128, 1], F32, name="zbias", tag="zbias")
        ones_c = persist.tile([128, 1], BF16, name="ones_c", tag="ones_c")
        kt0_sb = persist.tile([128, T], BF16, name="kt0_sb", tag="kt0_sb")
        mask_t = {
            g: persist.tile(
                [128, 4, mhi[g] - mlo[g]], BF16, name=f"mask{g}", tag=f"mask{g}"
            )
            for g in mask_d
        }
        dram = ctx.enter_context(tc.tile_pool(name="dram", bufs=1, space="DRAM"))
        ktd = dram.tile([NHG, 128, T], BF16, name="ktd")
        ktd_r = ktd.rearrange("j p t -> p j t")

        nc.vector.memset(zbias[:], 0.0)
        nc.vector.memset(ones_c[:], 1.0)
        # dummy broadcast so GPSIMD's LOAD_LIB (~7us) happens during the
        # startup DMA window, not at the first phase-B norm
        gpwarm = persist.tile([128, 1], F32, name="gpwarm", tag="gpwarm")
        nc.gpsimd.partition_broadcast(gpwarm[:], zbias[0:1, :])

        inv_s = 1.0 / float(np.sqrt(HD))

        # ---- phase A: projections, d-chunk-outer -------------------
        with (
            nc.named_scope("phase_A"),
            tc.tile_pool(name="wkp", bufs=1) as wkp,
            tc.tile_pool(name="xtp", bufs=2) as xtp,
            tc.tile_pool(name="kst", bufs=2) as kstp,
            tc.tile_pool(name="pa", bufs=8, space="PSUM") as pap,
        ):
            wk_sb = wkp.tile([128, ND, DG], BF16, name="wk", tag="wk")
            xt_sb0 = xtp.tile([128, ND, KTS], BF16, name="xt", tag="xt")

            # A-Q: Qt[j] = ((xq @ wq_j + bq_j)/sqrt(hd))^T, d-outer.
            # wq loads one 4-head column half at a time (the jg=1 half
            # reuses the tile once jg=0's matmuls retire).
            with (
                nc.named_scope("phase_AQ"),
                tc.tile_pool(name="wqp", bufs=1) as wqp,
                tc.tile_pool(name="xqp", bufs=1) as xqp,
            ):
                xq_sb = xqp.tile([128, ND, M], BF16, name="xq", tag="xq")
                wq_sb = wqp.tile([128, ND, 512], BF16, name="wq", tag="wq")
                # issue order = consumption order; first chunks split finest
                # so the PE starts on 256KB, and the wq stream goes through
                # the scalar engine's DGE queue so descriptor generation for
                # the two streams runs in parallel at startup
                nc.sync.dma_start(wq_sb[:, 0:1, :], wqp_d[:, 0:1, 0:512])
                nc.scalar.dma_start(xq_sb[:, 0:1, :], xqp_d[:, 0:1, :])
                nc.sync.dma_start(wq_sb[:, 1:2, :], wqp_d[:, 1:2, 0:512])
                nc.scalar.dma_start(xq_sb[:, 1:2, :], xqp_d[:, 1:2, :])
                nc.sync.dma_start(xq_sb[:, 2:4, :], xqp_d[:, 2:4, :])
                nc.scalar.dma_start(wq_sb[:, 2:4, :], wqp_d[:, 2:4, 0:512])
                nc.sync.dma_start(bias_q[:], bqs[:])
                nc.sync.dma_start(bias_k[:], bks[:])
                for g in range(1, 4):
                    nc.sync.dma_start(
                        xq_sb[:, 4 * g: 4 * g + 4, :], xqp_d[:, 4 * g: 4 * g + 4, :]
                    )
                    nc.scalar.dma_start(
                        wq_sb[:, 4 * g: 4 * g + 4, :],
                        wqp_d[:, 4 * g: 4 * g + 4, 0:512],
                    )
                pq = [pap.tile([128, M], F32, name="pq", tag="pa") for _ in range(NHG)]
                for jg in range(2):
                    for d in range(ND):
                        for jj in range(4):
                            nc.tensor.matmul(
                                pq[4 * jg + jj][:],
                                wq_sb[:, d, jj * 128: (jj + 1) * 128],
                                xq_sb[:, d, :],
                                start=(d == 0),
                                stop=(d == ND - 1),
                                skip_group_check=True,
                            )
                    for jj in range(4):
                        j = 4 * jg + jj
                        nc.scalar.activation(
                            qt_t[j][:],
                            pq[j][:],
                            mybir.ActivationFunctionType.Identity,
                            scale=inv_s,
                            bias=bias_q[:, j: j + 1],
                        )
                    if jg == 0:
                        # wq high half reloads the same tile: emitted after
                        # the jg=0 matmuls (WAR clears group by group) but
                        # ahead of wk/xt0 in the DMA queues
                        for g in range(4):
                            nc.sync.dma_start(
                                wq_sb[:, 4 * g: 4 * g + 4, :],
                                wqp_d[:, 4 * g: 4 * g + 4, 512:DG],
                            )
                        for g in range(4):
                            nc.sync.dma_start(
                                wk_sb[:, 4 * g: 4 * g + 4, 0:512],
                                wkp_d[:, 4 * g: 4 * g + 4, 0:512],
                            )
                            nc.sync.dma_start(
                                xt_sb0[:, 4 * g: 4 * g + 4, :],
                                xtp_d[7, :, 4 * g: 4 * g + 4, :],
                            )
                        for g in range(4):
                            nc.sync.dma_start(
                                wk_sb[:, 4 * g: 4 * g + 4, 512:DG],
                                wkp_d[:, 4 * g: 4 * g + 4, 512:DG],
                            )

            # A-KV: one pass over x computing Kt and V, d-outer groups.
            # wv lands in the space freed by wq/xq during K of ts=0.
            with (
                nc.named_scope("phase_AKV"),
                tc.tile_pool(name="wvp", bufs=1) as wvp,
            ):
                wv_sb = wvp.tile([128, ND, DG], BF16, name="wv", tag="wv")
                for g in range(4):
                    nc.sync.dma_start(
                        wv_sb[:, 4 * g: 4 * g + 4, :], wvp_d[:, 4 * g: 4 * g + 4, :]
                    )
                for g in mask_d:
                    nc.sync.dma_start(mask_t[g][:], mask_d[g][:])
                # ts=7 first: head 0's first lagged drains in phase B read
                # v_t[28:32], so those tiles must not be A's last DVE writes
                for tsi, ts in enumerate([7] + list(range(7))):
                    if tsi == 0:
                        xt_sb = xt_sb0
                    else:
                        xt_sb = xtp.tile([128, ND, KTS], BF16, name="xt", tag="xt")
                        nc.sync.dma_start(xt_sb[:], xtp_d[ts])
                    # K^T: two groups of 4 heads, 4 banks each
                    for jg in range(2):
                        pk = [pap.tile([128, KTS], F32, name="pk", tag="pa") for _ in range(4)]
                        for d in range(ND):
                            for jj in range(4):
                                j = 4 * jg + jj
                                nc.tensor.matmul(
                                    pk[jj][:],
                                    wk_sb[:, d, j * 128: (j + 1) * 128],
                                    xt_sb[:, d, :],
                                    start=(d == 0),
                                    stop=(d == ND - 1),
                                    skip_group_check=True,
                                )
                        ks = kstp.tile([128, 4, KTS], BF16, name="ks", tag="ks")
                        for jj in range(4):
                            j = 4 * jg + jj
                            nc.scalar.activation(
                                kt0_sb[:, ts * KTS: (ts + 1) * KTS] if j == 0
                                else ks[:, jj, :],
                                pk[jj][:],
                                mybir.ActivationFunctionType.Identity,
                                bias=bias_k[:, j: j + 1],
                            )
                        nc.sync.dma_start(
                            ktd_r[:, 4 * jg: 4 * jg + 4, ts * KTS: (ts + 1) * KTS],
                            ks[:],
                        )
                    # V: two f-groups of 4 t-subchunks, 4 banks each
                    for f in range(2):
                        pv = [pap.tile([128, 512], F32, name="pv", tag="pa") for _ in range(4)]
                        for d in range(ND):
                            for u in range(4):
                                nc.tensor.matmul(
                                    pv[u][:],
                                    xt_sb[:, d, u * 128: (u + 1) * 128],
                                    wv_sb[:, d, f * 512: (f + 1) * 512],
                                    start=(d == 0),
                                    stop=(d == ND - 1),
                                    skip_group_check=True,
                                )
                        for u in range(4):
                            nc.vector.tensor_copy(
                                v_t[ts * 4 + u][:, f * 512: (f + 1) * 512], pv[u][:]
                            )

        # ---- phase B prefetch: out-proj weights --------------------
        # (issued lazily inside phase B, 1MB per head boundary, so the 4MB
        # doesn't hog HBM bandwidth while the kth prefetches race B's start)
        wop = ctx.enter_context(tc.tile_pool(name="wop", bufs=1))
        wo_sb = wop.tile([128, NHG, D], BF16, name="wo", tag="wo")

        # ---- phase B: attention per head -----------------------------
        # Wide chunks (lo < 256) go through 2-bank pair supertiles with a
        # 2-buffer pool (distance-4 recycle hides the big exps); narrow
        # chunks (lo >= 256) are batched as QUADS into one single-buffered
        # 2-bank supertile: ONE exp + ONE mask-mul per 4 chunks, cutting the
        # 352-cycle fixed ACTIVATE cost (12 exps/head instead of 16).
        # A batch is [wide, narrow-quad, wide]: widest wide pairs with the
        # narrowest quad, so per-batch ACT load is even and the two PSUM
        # pools ping-pong.
        wide_pairs, narrow_quads = [], []
        for g in range(NT // 4):
            q = [i for i in range(4 * g, 4 * g + 4) if flo[i] < M]
            if not q:
                continue
            if (min(flo[i] for i in q) & ~3) >= 256:
                narrow_quads.append(q)
            else:
                wide_pairs += [q[k: k + 2] for k in (0, 2) if q[k: k + 2]]

        def _mk_batches(narrow_first):
            ws = list(wide_pairs)            # widest first
            ns = list(narrow_quads)[::-1]    # narrowest first
            batches = []
            while ws or ns:
                b = []
                if ws:
                    b.append(("w", ws.pop(0)))
                if ns:
                    b.append(("n", ns.pop(0)))
                if ws:
                    b.append(("w", ws.pop(0)))
                if narrow_first and len(b) > 1 and b[0][0] == "w" and b[1][0] == "n":
                    b[0], b[1] = b[1], b[0]
                batches.append(b)
            return batches

        batches_by_head = [_mk_batches(j == 0) for j in range(NHG)]
        with (
            nc.named_scope("phase_B"),
            tc.tile_pool(name="kth", bufs=3) as kthp,
            tc.tile_pool(name="ps", bufs=2, space="PSUM") as psp,
            tc.tile_pool(name="psn", bufs=1, space="PSUM") as psnp,
            tc.tile_pool(name="po", bufs=1, space="PSUM") as pop,
            tc.tile_pool(name="pl", bufs=1, space="PSUM") as plp,
            tc.tile_pool(name="esb", bufs=5) as esb,
            tc.tile_pool(name="lsb", bufs=1) as lsb,
        ):
            po_q, pl_q = {}, {}
            # both row-sum accumulators share one bank (partitions 0/32)
            pl_t = plp.tile([64, M], F32, name="pl", tag="pl")

            def emit_norm(j):
                """Normalize head j: ot[j] = po[j] / l[j] (off PE critical path).

                The reciprocal row is broadcast across partitions on GPSIMD
                (idle engine) instead of a PE matmul + ACT copy, freeing a
                PSUM bank for the score pipeline.
                """
                po, pl = po_q.pop(j), pl_q.pop(j)
                l_sb = lsb.tile([1, M], F32, name="l", tag="l")
                linv = lsb.tile([1, M], F32, name="linv", tag="linv")
                nc.vector.tensor_copy(l_sb[:], pl)
                nc.vector.reciprocal_approx_fast(linv[:], l_sb[:])
                lb_sb = lsb.tile([128, M], F32, name="lb", tag="lb")
                nc.gpsimd.partition_broadcast(lb_sb[:], linv[:])
                nc.vector.tensor_mul(ot_t[j][:], po[:], lb_sb[:])

            kth = {0: kt0_sb}
            issued = {0}

            def ensure_kth(jn):
                if jn < NHG and jn not in issued:
                    kth[jn] = kthp.tile([128, T], BF16, name="kth", tag="kth")
                    nc.sync.dma_start(kth[jn][:], ktd[jn])
                    issued.add(jn)

            state = {}  # j -> [po_start_pending, l_start_pending]

            def drain(pj, cur, rsl, last_b):
                """Emit the lagged PV + row-sum matmuls for head pj's batch."""
                if pj not in po_q:
                    # lazy: with norm emitted before the first drain of the
                    # next head, only one PV accumulator bank is live
                    po_q[pj] = pop.tile([128, M], F32, name="po", tag="po")
                st = state[pj]
                for k, (pair, e, uoff, lo) in enumerate(cur):
                    for u, i in enumerate(pair):
                        nc.tensor.matmul(
                            po_q[pj][:, lo:M],
                            v_t[i][:, pj * 128: (pj + 1) * 128],
                            e[:, uoff + u, lo:M],
                            start=st[0],
                            stop=(last_b and k == len(cur) - 1 and u == len(pair) - 1),
                            skip_group_check=True,
                        )
                        st[0] = False
                for k, (es2, uoff, lo) in enumerate(rsl):
                    rs = es2[:, lo:M] if uoff is None else es2[:, uoff, lo:M]
                    nc.tensor.matmul(
                        pl_q[pj][:, lo:M], ones_c[:], rs,
                        start=st[1], stop=(last_b and k == len(rsl) - 1),
                        skip_group_check=True,
                    )
                    st[1] = False

            # flat (head, batch) pipeline: PV/l lag the S/exp stream by TWO
            # batches ACROSS head boundaries, so the PE always has
            # dependency-free drain work to absorb exp latency
            pendq = []  # [(j, cur, is_last_batch_of_head)]
            for j in range(NHG):
                ensure_kth(j + 1)
                if j < 4:
                    # out-proj weights trickle in 1MB/head behind the kth
                    # prefetches; only needed at phase C
                    nc.sync.dma_start(
                        wo_sb[:, 2 * j: 2 * j + 2, :], wop_d[:, 2 * j: 2 * j + 2, :]
                    )
                pl_q[j] = pl_t[32 * (j % 2): 32 * (j % 2) + 1, :]
                state[j] = [True, True]
                batches = batches_by_head[j]
                for bi, batch in enumerate(batches):
                    # drain first: with the lag-3 window this finishes head
                    # j-1's PV/l accumulation by bi==2, a full batch before
                    # its po bank partner is written again
                    if len(pendq) >= 3:
                        drain(*pendq.pop(0))
                    if bi == 2 and j > 0 and (j - 1) in po_q:
                        emit_norm(j - 1)
                    cur = []   # PV entries: (pair, e, uoff, lo_pv)
                    rsl = []   # row-sum entries: (tile, uoff|None, lo)
                    for kind, grp in batch:
                        g = grp[0] // 4
                        ng = len(grp)
                        lo_g = min(flo[i] for i in grp) & ~3
                        if kind == "n":
                            pst = psnp.tile([128, 4, 256], F32, name="pstn", tag="psn")
                            off = 256
                        else:
                            pst = psp.tile([128, 2, M], F32, name="pst", tag="ps")
                            off = 0
                        for u, i in enumerate(grp):
                            nc.tensor.matmul(
                                pst[:, u, lo_g - off: M - off],
                                kth[j][:, i * 128: (i + 1) * 128],
                                qt_t[j][:, lo_g:M],
                                start=True,
                                stop=True,
                                skip_group_check=True,
                            )
                        # ONE exp straight off PSUM for the whole group
                        # (frees the score tile), then ONE multiplicative 0/1
                        # causal mask over the partial band
                        if kind == "n":
                            e = esb.tile([128, 4, M], BF16, name="e4", tag="e4", bufs=5)
                        else:
                            e = esb.tile([128, 2, M], BF16, name="e2", tag="e2", bufs=8)
                        nc.scalar.activation(
                            e[:, :ng, lo_g:M],
                            pst[:, :ng, lo_g - off: M - off],
                            mybir.ActivationFunctionType.Exp,
                            bias=zbias[:],
                        )
                        fhi_max = max(fhi[i] for i in grp)
                        um = grp[0] % 4
                        if lo_g < fhi_max:
                            nc.vector.tensor_mul(
                                e[:, :ng, lo_g:fhi_max],
                                e[:, :ng, lo_g:fhi_max],
                                mask_t[g][:, um: um + ng, lo_g - mlo[g]: fhi_max - mlo[g]],
                            )
                        for uo in range(0, ng, 2):
                            pair = grp[uo: uo + 2]
                            lo_pv = min(flo[i] for i in pair) & ~3
                            if len(pair) == 2:
                                # pair-sum on DVE so the PE does one row-sum
                                # matmul per pair instead of per chunk
                                esum = esb.tile(
                                    [128, M], BF16, name="esum", tag="esum", bufs=14
                                )
                                nc.vector.tensor_add(
                                    esum[:, lo_pv:M],
                                    e[:, uo, lo_pv:M],
                                    e[:, uo + 1, lo_pv:M],
                                )
                                rsl.append((esum, None, lo_pv))
                            else:
                                rsl.append((e, uo, lo_pv))
                            cur.append((pair, e, uo, lo_pv))
                    pendq.append((j, cur, rsl, bi == len(batches) - 1))
                    if bi == 1:
                        ensure_kth(j + 2)
            for p in pendq:
                drain(*p)
            emit_norm(NHG - 1)

        # ---- phase C: y = O @ wo  (row-parallel partial, bf16 out) -
        with (
            nc.named_scope("phase_C"),
            tc.tile_pool(name="py", bufs=2, space="PSUM") as pyp,
            tc.tile_pool(name="ysb", bufs=2) as ysb,
        ):
            # output DMAs alternate across the two DGE queues so the 2MB of
            # y doesn't serialize on one ring at the very end; the last
            # block ships in two halves to shorten the post-matmul tail
            for mb in range(M // 128):
                ys = ysb.tile([128, D], BF16, name="ys", tag="ys")
                last = mb == M // 128 - 1
                dma_eng = nc.sync if mb % 2 == 0 else nc.scalar
                for fp in range(D // 1024):
                    py = [
                        pyp.tile([128, 512], F32, name="py", tag=f"py{2 * (fp % 2) + h}")
                        for h in range(2)
                    ]
                    for j in range(NHG):
                        for h in range(2):
                            fo = 2 * fp + h
                            nc.tensor.matmul(
                                py[h][:],
                                ot_t[j][:, mb * 128: (mb + 1) * 128],
                                wo_sb[:, j, fo * 512: (fo + 1) * 512],
                                start=(j == 0),
                                stop=(j == NHG - 1),
                                skip_group_check=True,
                            )
                    for h in range(2):
                        fo = 2 * fp + h
                        nc.scalar.copy(ys[:, fo * 512: (fo + 1) * 512], py[h][:])
                        if last:
                            dma_eng = nc.sync if fo % 2 == 0 else nc.scalar
                            dma_eng.dma_start(
                                y[mb][:, fo * 512: (fo + 1) * 512],
                                ys[:, fo * 512: (fo + 1) * 512],
                            )
                if not last:
                    dma_eng.dma_start(y[mb], ys[:])

    nc.compile()
    return nc


_cache = {}


def _get_program(flo, fhi):
    key = (tuple(flo), tuple(fhi))
    if key not in _cache:
        _cache[key] = build_program(list(flo), list(fhi))
    return _cache[key]


def _packc(a):
    """[C*128, N] row-major -> [128][C][N]: per-partition contiguous runs."""
    c = a.shape[0] // 128
    return np.ascontiguousarray(a.reshape(c, 128, a.shape[1]).transpose(1, 0, 2))


def _prep(inputs):
    x = np.asarray(inputs["x"], dtype=np.float32)
    qidx = np.asarray(inputs["query_idx"]).astype(np.int64)
    Wq = np.asarray(inputs["Wq"], dtype=np.float32)
    Wk = np.asarray(inputs["Wk"], dtype=np.float32)
    Wv = np.asarray(inputs["Wv"], dtype=np.float32)
    Wo = np.asarray(inputs["Wo"], dtype=np.float32)
    bq = np.asarray(inputs["bq"], dtype=np.float32)
    bk = np.asarray(inputs["bk"], dtype=np.float32)
    bv = np.asarray(inputs["bv"], dtype=np.float32)
    bo = np.asarray(inputs["bo"], dtype=np.float32)

    # Per-t-chunk skip bounds, union over batches.  flo[i] = first m that
    # attends into chunk i (everything below is fully masked there);
    # fhi[i] = one past the last m only partially covered by chunk i.
    # Computed positionally so they are correct even for unsorted
    # query_idx (just less effective at skipping).
    flo = [M] * NT
    fhi = [0] * NT
    for b in range(B):
        for i in range(NT):
            allowed = qidx[b] >= 128 * i          # chunk i not fully masked
            partial = qidx[b] < 128 * (i + 1)     # chunk i not fully allowed
            lo_b = int(np.argmax(allowed)) if allowed.any() else M
            hi_b = M - int(np.argmax(partial[::-1])) if partial.any() else 0
            flo[i] = min(flo[i], lo_b)
            fhi[i] = max(fhi[i], hi_b)
    mlo = [min(flo[4 * g: 4 * g + 4]) & ~3 for g in range(NT // 4)]
    mhi = [max(fhi[4 * g: 4 * g + 4]) for g in range(NT // 4)]

    in_maps = []
    tgrid = np.arange(T)[:, None]
    for core in range(8):
        b, g = divmod(core, 2)
        sl = slice(g * DG, (g + 1) * DG)
        xb = x[b]
        xT = xb.T.astype(NPBF)                                # [D, T]
        # [ts][128][ND][KTS]: per-partition 16KB contiguous runs
        xtp = np.ascontiguousarray(
            xT.reshape(ND, 128, T // KTS, KTS).transpose(2, 1, 0, 3)
        )
        # 0/1 multiplicative causal mask
        mask = (tgrid <= qidx[b][None, :]).astype(NPBF)
        mask4 = mask.reshape(NT, 128, M)
        im = {
            "xtp": xtp,
            "xqp": _packc(xb[qidx[b]].T.astype(NPBF)),
            "wkp": _packc(Wk[:, sl].astype(NPBF)),
            "wvp": _packc(Wv[:, sl].astype(NPBF)),
            "wqp": _packc(Wq[:, sl].astype(NPBF)),
            "wop": _packc(Wo[sl, :].astype(NPBF)),
            "bks": np.ascontiguousarray(bk[sl].reshape(NHG, 128).T),
            "bqs": np.ascontiguousarray(
                (bq[sl] / np.sqrt(HD)).reshape(NHG, 128).T.astype(np.float32)
            ),
        }
        for g4 in range(NT // 4):
            if mlo[g4] < M and mhi[g4] > mlo[g4]:
                im[f"mask{g4}"] = np.ascontiguousarray(
                    mask4[4 * g4: 4 * g4 + 4, :, mlo[g4]: mhi[g4]].transpose(1, 0, 2)
                )
        in_maps.append(im)

    const = (bv.astype(np.float64) @ Wo.astype(np.float64) + bo).astype(np.float32)
    return flo, fhi, in_maps, const


def run(inputs, trace=False, trace_kwargs=None):
    _install_ntff_hook()
    flo, fhi, in_maps, const = _prep(inputs)
    nc = _get_program(flo, fhi)
    res = run_bass_kernel_spmd(
        nc, in_maps, list(range(8)), trace=trace, **(trace_kwargs or {})
    )
    out = np.zeros((B, M, D), dtype=np.float32)
    for b in range(B):
        out[b] = (
            res.results[2 * b]["y"].reshape(M, D).astype(np.float32)
            + res.results[2 * b + 1]["y"].reshape(M, D).astype(np.float32)
            + const
        )
    return out, res


def kernel(**inputs) -> np.ndarray:
    out, _ = run(inputs, trace=False)
    return out



# revision 47
# speedup vs baseline: 1.2492x; 1.0436x over previous
"""Trainium2 Bass kernel for sparse causal attention (nn_CausalAttentionKV).

Reference computation (fp32, single device):
    q_all = x @ Wq + bq ; k_all = x @ Wk + bk ; v_all = x @ Wv + bv
    q = gather(q_all, query_idx)        # (B, M, D) selected query rows
    att = softmax(mask(q k^T / sqrt(hd)))   # per-query causal mask t <= qidx[m]
    y = (att v) @ Wo + bo

Shapes: B=4, T=4096, D=2048, n_head=16, hd=128, M=512.

Sharding (8 cores): core = 2*b + g  handles batch b and head-group g
(8 heads = 1024 feature cols).  Q/K/V projections are column-parallel,
out-proj is row-parallel; the two partial outputs per batch are summed
on the host.  All matmul inputs are bf16 (fp32 PSUM accumulation).

Scheduling notes (measured 630 us HW vs 663 us for the previous
version; ~566 us is the bf16 PE streaming floor for this split):
- dma_start costs ~0.6us of sync-engine descriptor generation each, so
  every operand is one SBUF supertile filled by a handful of large
  fully-contiguous transfers (host pre-packs [partition][contig]
  layouts), issue-ordered to match consumption.  Reusing-tile reloads
  (wq high half) must be EMITTED after the full consumer group
  including its PSUM-draining activations, or results corrupt.
- Phase A runs d-chunk-outer: Q, then K/V per 512-t chunk, rotating
  4-bank groups through one 8-buffer PSUM pool; the PE starts ~13us in
  (7us framework prologue + first 512KB) and never drains at the
  A-Q -> A-KV seam.
- The causal mask is applied MULTIPLICATIVELY (0/1) to e = exp(s) on
  SBUF after the activation, not additively on PSUM before it: the
  PSUM score tile is freed by the exp itself, shortening the
  PE -> ACT recycle chain that throttled the attention pipeline.
- Phase B: both softmax row-sum accumulators share one PSUM bank
  (partitions 0/32), the norm broadcast has its own bank, K^T is
  prefetched two heads ahead (3 buffers), wide/narrow chunk pairs are
  interleaved so the big exps don't cluster, the PV/row-sum drain lags
  the score/exp stream by TWO batches (dependency-free PE filler), and
  all column windows are 4-aligned for the DVE/ACT wide modes.
- Phase C writes bf16 partial outputs through one staging tile per
  128-row block (4 dma_starts total); the host sums the two partials
  per batch in fp32.
"""

import sys
import types
from contextlib import ExitStack

import numpy as np
import ml_dtypes

import concourse.bass as bass
import concourse.tile as tile
import concourse.mybir as mybir
from concourse import bacc
from concourse.bass_utils import run_bass_kernel_spmd

BF16 = mybir.dt.bfloat16
F32 = mybir.dt.float32
F8 = mybir.dt.float8e4
NPBF = ml_dtypes.bfloat16
NPF8 = ml_dtypes.float8_e4m3   # TRN FP8_EXP4-compatible (max ±240)

B, T, D = 4, 4096, 2048
NH, HD, M = 16, 128, 512
NHG = 8            # heads per core (group)
DG = NHG * HD      # 1024 feature cols per core
NT = T // 128      # 32 t-chunks
ND = D // 128      # 16 d-chunks
KTS = 512
# K-projection fp8 heads: these heads' K is computed with e4m3 DoubleRow
# matmuls (256-deep contraction, ~1.7x PE rate).  Numerics sim: 2 fp8
# heads/core -> rel_err 1.72e-2 vs the 2e-2 gate (bf16 baseline 5.5e-3).
FP8H = (3, 7)
W8S = 32.0         # Wk is scaled x32 before e4m3 (std 0.022 -> 0.7)


def _install_ntff_hook():
    """Register the axon NTFF profiling hook if the image's antenv lacks it."""
    try:
        from antenv.axon_hooks import get_axon_ntff_profile_hook  # noqa: F401
        return
    except ImportError:
        pass
    try:
        import antenv
        from trn_agent_boot.trn_boot import _ntff_profile_via_ctypes

        mod = types.ModuleType("antenv.axon_hooks")
        hook = [None]
        mod.set_axon_ntff_profile_hook = lambda h: hook.__setitem__(0, h)
        mod.get_axon_ntff_profile_hook = lambda: hook[0]
        sys.modules["antenv.axon_hooks"] = mod
        antenv.axon_hooks = mod
        mod.set_axon_ntff_profile_hook(
            _ntff_profile_via_ctypes("/opt/axon/libaxon_pjrt.so")
        )
    except Exception:
        pass


def build_program(flo, fhi):
    """Build the per-core Bass program.

    flo[i]: first m column with any allowed key in t-chunk i (cols below
            are fully masked there -> never computed).
    fhi[i]: first m column fully allowed in t-chunk i (cols beyond need
            no mask).
    Both are unions over the 4 batches so one program serves all cores.
    """
    nc = bacc.Bacc("TRN2", target_bir_lowering=False, debug=False)

    # host-packed layouts: per-partition fully contiguous runs
    xtp_d = nc.dram_tensor("xtp", [T // KTS, 128, ND, KTS], BF16, kind="ExternalInput")
    xt8p_d = nc.dram_tensor("xt8p", [T // KTS, 128, ND, KTS], F8, kind="ExternalInput")
    wk8p_d = nc.dram_tensor(
        "wk8p", [128, ND, 128 * len(FP8H)], F8, kind="ExternalInput"
    )
    xqp_d = nc.dram_tensor("xqp", [128, ND, M], BF16, kind="ExternalInput")
    wkp_d = nc.dram_tensor("wkp", [128, ND, DG], BF16, kind="ExternalInput")
    wvp_d = nc.dram_tensor("wvp", [128, ND, DG], BF16, kind="ExternalInput")
    wqp_d = nc.dram_tensor("wqp", [128, ND, DG], BF16, kind="ExternalInput")
    wop_d = nc.dram_tensor("wop", [128, NHG, D], BF16, kind="ExternalInput")
    bks = nc.dram_tensor("bks", [128, NHG], F32, kind="ExternalInput")
    bqs = nc.dram_tensor("bqs", [128, NHG], F32, kind="ExternalInput")
    y = nc.dram_tensor("y", [M // 128, 128, D], BF16, kind="ExternalOutput")

    # 0/1 mask super-tiles: 4 t-chunks each, shared col-window.
    # mlo is 4-col aligned so every AP base lands on 8-byte boundaries
    # (keeps DVE/ACT in their wide modes).
    mlo = [min(flo[4 * g: 4 * g + 4]) & ~3 for g in range(NT // 4)]
    mhi = [max(fhi[4 * g: 4 * g + 4]) for g in range(NT // 4)]
    mask_d = {}
    for g in range(NT // 4):
        if mlo[g] < M and mhi[g] > mlo[g]:
            mask_d[g] = nc.dram_tensor(
                f"mask{g}", [128, 4, mhi[g] - mlo[g]], BF16, kind="ExternalInput"
            )

    with ExitStack() as ctx:
        tc = ctx.enter_context(tile.TileContext(nc))

        # ---- persistent tiles --------------------------------------
        persist = ctx.enter_context(tc.tile_pool(name="persist", bufs=1))
        v_t = [persist.tile([128, DG], BF16, name=f"v{i}", tag=f"v{i}") for i in range(NT)]
        qt_t = [persist.tile([128, M], BF16, name=f"qt{j}", tag=f"qt{j}") for j in range(NHG)]
        # ot overlays qt: head j's normalized output overwrites its query
        # tile after the last score matmul read it (WAR dep via Tile)
        ot_t = qt_t
        bias_k = persist.tile([128, NHG], F32, name="bias_k", tag="bias_k")
        bias_q = persist.tile([128, NHG], F32, name="bias_q", tag="bias_q")
        zbias = persist.tile([128, 1], F32, name="zbias", tag="zbias")
        ones_c = persist.tile([128, 1], BF16, name="ones_c", tag="ones_c")
        kt0_sb = persist.tile([128, T], BF16, name="kt0_sb", tag="kt0_sb")
        mask_t = {
            g: persist.tile(
                [128, 4, mhi[g] - mlo[g]], BF16, name=f"mask{g}", tag=f"mask{g}"
            )
            for g in mask_d
        }
        dram = ctx.enter_context(tc.tile_pool(name="dram", bufs=1, space="DRAM"))
        ktd = dram.tile([NHG, 128, T], BF16, name="ktd")
        ktd_r = ktd.rearrange("j p t -> p j t")

        nc.vector.memset(zbias[:], 0.0)
        nc.vector.memset(ones_c[:], 1.0)
        # dummy broadcast so GPSIMD's LOAD_LIB (~7us) happens during the
        # startup DMA window, not at the first phase-B norm
        gpwarm = persist.tile([128, 1], F32, name="gpwarm", tag="gpwarm")
        nc.gpsimd.partition_broadcast(gpwarm[:], zbias[0:1, :])

        inv_s = 1.0 / float(np.sqrt(HD))

        # ---- phase A: projections, d-chunk-outer -------------------
        with (
            nc.named_scope("phase_A"),
            tc.tile_pool(name="wkp", bufs=1) as wkp,
            tc.tile_pool(name="xtp", bufs=2) as xtp,
            tc.tile_pool(name="xt8p", bufs=1) as xt8p,
            tc.tile_pool(name="kst", bufs=2) as kstp,
            tc.tile_pool(name="pa", bufs=8, space="PSUM") as pap,
        ):
            wk_sb = wkp.tile([128, ND, DG], BF16, name="wk", tag="wk")
            wk8_sb = wkp.tile([128, ND, 128 * len(FP8H)], F8, name="wk8", tag="wk8")
            xt_sb0 = xtp.tile([128, ND, KTS], BF16, name="xt", tag="xt")
            xt8_sb0 = xt8p.tile([128, ND, KTS], F8, name="xt8", tag="xt8")

            # A-Q: Qt[j] = ((xq @ wq_j + bq_j)/sqrt(hd))^T, d-outer.
            # wq loads one 4-head column half at a time (the jg=1 half
            # reuses the tile once jg=0's matmuls retire).
            with (
                nc.named_scope("phase_AQ"),
                tc.tile_pool(name="wqp", bufs=1) as wqp,
                tc.tile_pool(name="xqp", bufs=1) as xqp,
            ):
                xq_sb = xqp.tile([128, ND, M], BF16, name="xq", tag="xq")
                wq_sb = wqp.tile([128, ND, 512], BF16, name="wq", tag="wq")
                # issue order = consumption order; first chunks split finest
                # so the PE starts on 256KB, and the wq stream goes through
                # the scalar engine's DGE queue so descriptor generation for
                # the two streams runs in parallel at startup
                nc.sync.dma_start(wq_sb[:, 0:1, :], wqp_d[:, 0:1, 0:512])
                nc.scalar.dma_start(xq_sb[:, 0:1, :], xqp_d[:, 0:1, :])
                nc.sync.dma_start(wq_sb[:, 1:2, :], wqp_d[:, 1:2, 0:512])
                nc.scalar.dma_start(xq_sb[:, 1:2, :], xqp_d[:, 1:2, :])
                nc.sync.dma_start(xq_sb[:, 2:4, :], xqp_d[:, 2:4, :])
                nc.scalar.dma_start(wq_sb[:, 2:4, :], wqp_d[:, 2:4, 0:512])
                nc.sync.dma_start(bias_q[:], bqs[:])
                nc.sync.dma_start(bias_k[:], bks[:])
                for g in range(1, 4):
                    nc.sync.dma_start(
                        xq_sb[:, 4 * g: 4 * g + 4, :], xqp_d[:, 4 * g: 4 * g + 4, :]
                    )
                    nc.scalar.dma_start(
                        wq_sb[:, 4 * g: 4 * g + 4, :],
                        wqp_d[:, 4 * g: 4 * g + 4, 0:512],
                    )
                pq = [pap.tile([128, M], F32, name="pq", tag="pa") for _ in range(NHG)]
                for jg in range(2):
                    for d in range(ND):
                        for jj in range(4):
                            nc.tensor.matmul(
                                pq[4 * jg + jj][:],
                                wq_sb[:, d, jj * 128: (jj + 1) * 128],
                                xq_sb[:, d, :],
                                start=(d == 0),
                                stop=(d == ND - 1),
                                skip_group_check=True,
                            )
                    for jj in range(4):
                        j = 4 * jg + jj
                        nc.scalar.activation(
                            qt_t[j][:],
                            pq[j][:],
                            mybir.ActivationFunctionType.Identity,
                            scale=inv_s,
                            bias=bias_q[:, j: j + 1],
                        )
                    if jg == 0:
                        # wq high half reloads the same tile: emitted after
                        # the jg=0 matmuls (WAR clears group by group) but
                        # ahead of wk/xt0 in the DMA queues
                        for g in range(4):
                            nc.sync.dma_start(
                                wq_sb[:, 4 * g: 4 * g + 4, :],
                                wqp_d[:, 4 * g: 4 * g + 4, 512:DG],
                            )
                        for g in range(4):
                            nc.sync.dma_start(
                                wk_sb[:, 4 * g: 4 * g + 4, 0:512],
                                wkp_d[:, 4 * g: 4 * g + 4, 0:512],
                            )
                            nc.sync.dma_start(
                                xt_sb0[:, 4 * g: 4 * g + 4, :],
                                xtp_d[7, :, 4 * g: 4 * g + 4, :],
                            )
                        for g in range(4):
                            nc.sync.dma_start(
                                wk_sb[:, 4 * g: 4 * g + 4, 512:DG],
                                wkp_d[:, 4 * g: 4 * g + 4, 512:DG],
                            )
                        nc.sync.dma_start(wk8_sb[:], wk8p_d[:])
                        nc.scalar.dma_start(xt8_sb0[:], xt8p_d[7])

            # A-KV: one pass over x computing Kt and V, d-outer groups.
            # wv lands in the space freed by wq/xq during K of ts=0.
            with (
                nc.named_scope("phase_AKV"),
                tc.tile_pool(name="wvp", bufs=1) as wvp,
            ):
                wv_sb = wvp.tile([128, ND, DG], BF16, name="wv", tag="wv")
                for g in range(4):
                    nc.sync.dma_start(
                        wv_sb[:, 4 * g: 4 * g + 4, :], wvp_d[:, 4 * g: 4 * g + 4, :]
                    )
                for g in mask_d:
                    nc.sync.dma_start(mask_t[g][:], mask_d[g][:])
                # ts=7 first: head 0's first lagged drains in phase B read
                # v_t[28:32], so those tiles must not be A's last DVE writes
                for tsi, ts in enumerate([7] + list(range(7))):
                    if tsi == 0:
                        xt_sb = xt_sb0
                        xt8_sb = xt8_sb0
                    else:
                        xt_sb = xtp.tile([128, ND, KTS], BF16, name="xt", tag="xt")
                        nc.sync.dma_start(xt_sb[:], xtp_d[ts])
                        xt8_sb = xt8p.tile([128, ND, KTS], F8, name="xt8", tag="xt8")
                        nc.scalar.dma_start(xt8_sb[:], xt8p_d[ts])
                    # K^T: two groups of 4 heads, 4 banks each; FP8H heads
                    # run e4m3 DoubleRow (256-deep contraction per matmul)
                    for jg in range(2):
                        pk = [pap.tile([128, KTS], F32, name="pk", tag="pa") for _ in range(4)]
                        for d in range(ND):
                            for jj in range(4):
                                j = 4 * jg + jj
                                if j in FP8H:
                                    continue
                                nc.tensor.matmul(
                                    pk[jj][:],
                                    wk_sb[:, d, j * 128: (j + 1) * 128],
                                    xt_sb[:, d, :],
                                    start=(d == 0),
                                    stop=(d == ND - 1),
                                    skip_group_check=True,
                                )
                        for jj in range(4):
                            j = 4 * jg + jj
                            if j not in FP8H:
                                continue
                            c0 = FP8H.index(j) * 128
                            for dp in range(ND // 2):
                                nc.tensor.matmul(
                                    pk[jj][:],
                                    wk8_sb[:, 2 * dp: 2 * dp + 2, c0: c0 + 128],
                                    xt8_sb[:, 2 * dp: 2 * dp + 2, :],
                                    start=(dp == 0),
                                    stop=(dp == ND // 2 - 1),
                                    perf_mode=mybir.MatmulPerfMode.DoubleRow,
                                    skip_group_check=True,
                                )
                        ks = kstp.tile([128, 4, KTS], BF16, name="ks", tag="ks")
                        for jj in range(4):
                            j = 4 * jg + jj
                            nc.scalar.activation(
                                kt0_sb[:, ts * KTS: (ts + 1) * KTS] if j == 0
                                else ks[:, jj, :],
                                pk[jj][:],
                                mybir.ActivationFunctionType.Identity,
                                scale=(1.0 / W8S) if j in FP8H else 1.0,
                                bias=bias_k[:, j: j + 1],
                            )
                        nc.sync.dma_start(
                            ktd_r[:, 4 * jg: 4 * jg + 4, ts * KTS: (ts + 1) * KTS],
                            ks[:],
                        )
                    # V: two f-groups of 4 t-subchunks, 4 banks each
                    for f in range(2):
                        pv = [pap.tile([128, 512], F32, name="pv", tag="pa") for _ in range(4)]
                        for d in range(ND):
                            for u in range(4):
                                nc.tensor.matmul(
                                    pv[u][:],
                                    xt_sb[:, d, u * 128: (u + 1) * 128],
                                    wv_sb[:, d, f * 512: (f + 1) * 512],
                                    start=(d == 0),
                                    stop=(d == ND - 1),
                                    skip_group_check=True,
                                )
                        for u in range(4):
                            nc.vector.tensor_copy(
                                v_t[ts * 4 + u][:, f * 512: (f + 1) * 512], pv[u][:]
                            )

        # ---- phase B prefetch: out-proj weights --------------------
        # (issued lazily inside phase B, 1MB per head boundary, so the 4MB
        # doesn't hog HBM bandwidth while the kth prefetches race B's start)
        wop = ctx.enter_context(tc.tile_pool(name="wop", bufs=1))
        wo_sb = wop.tile([128, NHG, D], BF16, name="wo", tag="wo")

        # ---- phase B: attention per head -----------------------------
        # Wide chunks (lo < 256) go through 2-bank pair supertiles with a
        # 2-buffer pool (distance-4 recycle hides the big exps); narrow
        # chunks (lo >= 256) are batched as QUADS into one single-buffered
        # 2-bank supertile: ONE exp + ONE mask-mul per 4 chunks, cutting the
        # 352-cycle fixed ACTIVATE cost (12 exps/head instead of 16).
        # A batch is [wide, narrow-quad, wide]: widest wide pairs with the
        # narrowest quad, so per-batch ACT load is even and the two PSUM
        # pools ping-pong.
        wide_pairs, narrow_quads = [], []
        for g in range(NT // 4):
            q = [i for i in range(4 * g, 4 * g + 4) if flo[i] < M]
            if not q:
                continue
            if (min(flo[i] for i in q) & ~3) >= 256:
                narrow_quads.append(q)
            else:
                wide_pairs += [q[k: k + 2] for k in (0, 2) if q[k: k + 2]]

        def _mk_batches(narrow_first):
            ws = list(wide_pairs)            # widest first
            ns = list(narrow_quads)[::-1]    # narrowest first
            batches = []
            while ws or ns:
                b = []
                if ws:
                    b.append(("w", ws.pop(0)))
                if ns:
                    b.append(("n", ns.pop(0)))
                if ws:
                    b.append(("w", ws.pop(0)))
                if narrow_first and len(b) > 1 and b[0][0] == "w" and b[1][0] == "n":
                    b[0], b[1] = b[1], b[0]
                batches.append(b)
            return batches

        batches_by_head = [_mk_batches(j == 0) for j in range(NHG)]
        with (
            nc.named_scope("phase_B"),
            tc.tile_pool(name="kth", bufs=3) as kthp,
            tc.tile_pool(name="ps", bufs=2, space="PSUM") as psp,
            tc.tile_pool(name="psn", bufs=1, space="PSUM") as psnp,
            tc.tile_pool(name="po", bufs=1, space="PSUM") as pop,
            tc.tile_pool(name="pl", bufs=1, space="PSUM") as plp,
            tc.tile_pool(name="esb", bufs=5) as esb,
            tc.tile_pool(name="lsb", bufs=1) as lsb,
        ):
            po_q, pl_q = {}, {}
            # both row-sum accumulators share one bank (partitions 0/32)
            pl_t = plp.tile([64, M], F32, name="pl", tag="pl")

            def emit_norm(j):
                """Normalize head j: ot[j] = po[j] / l[j] (off PE critical path).

                The reciprocal row is broadcast across partitions on GPSIMD
                (idle engine) instead of a PE matmul + ACT copy, freeing a
                PSUM bank for the score pipeline.
                """
                po, pl = po_q.pop(j), pl_q.pop(j)
                l_sb = lsb.tile([1, M], F32, name="l", tag="l")
                linv = lsb.tile([1, M], F32, name="linv", tag="linv")
                nc.vector.tensor_copy(l_sb[:], pl)
                nc.vector.reciprocal_approx_fast(linv[:], l_sb[:])
                lb_sb = lsb.tile([128, M], F32, name="lb", tag="lb")
                nc.gpsimd.partition_broadcast(lb_sb[:], linv[:])
                nc.vector.tensor_mul(ot_t[j][:], po[:], lb_sb[:])

            kth = {0: kt0_sb}
            issued = {0}

            def ensure_kth(jn):
                if jn < NHG and jn not in issued:
                    kth[jn] = kthp.tile([128, T], BF16, name="kth", tag="kth")
                    nc.sync.dma_start(kth[jn][:], ktd[jn])
                    issued.add(jn)

            state = {}  # j -> [po_start_pending, l_start_pending]

            def drain(pj, cur, rsl, last_b):
                """Emit the lagged PV + row-sum matmuls for head pj's batch."""
                if pj not in po_q:
                    # lazy: with norm emitted before the first drain of the
                    # next head, only one PV accumulator bank is live
                    po_q[pj] = pop.tile([128, M], F32, name="po", tag="po")
                st = state[pj]
                for k, (pair, e, uoff, lo) in enumerate(cur):
                    for u, i in enumerate(pair):
                        nc.tensor.matmul(
                            po_q[pj][:, lo:M],
                            v_t[i][:, pj * 128: (pj + 1) * 128],
                            e[:, uoff + u, lo:M],
                            start=st[0],
                            stop=(last_b and k == len(cur) - 1 and u == len(pair) - 1),
                            skip_group_check=True,
                        )
                        st[0] = False
                for k, (es2, uoff, lo) in enumerate(rsl):
                    rs = es2[:, lo:M] if uoff is None else es2[:, uoff, lo:M]
                    nc.tensor.matmul(
                        pl_q[pj][:, lo:M], ones_c[:], rs,
                        start=st[1], stop=(last_b and k == len(rsl) - 1),
                        skip_group_check=True,
                    )
                    st[1] = False

            # flat (head, batch) pipeline: PV/l lag the S/exp stream by TWO
            # batches ACROSS head boundaries, so the PE always has
            # dependency-free drain work to absorb exp latency
            pendq = []  # [(j, cur, is_last_batch_of_head)]
            for j in range(NHG):
                ensure_kth(j + 1)
                if j < 4:
                    # out-proj weights trickle in 1MB/head behind the kth
                    # prefetches; only needed at phase C
                    nc.sync.dma_start(
                        wo_sb[:, 2 * j: 2 * j + 2, :], wop_d[:, 2 * j: 2 * j + 2, :]
                    )
                pl_q[j] = pl_t[32 * (j % 2): 32 * (j % 2) + 1, :]
                state[j] = [True, True]
                batches = batches_by_head[j]
                for bi, batch in enumerate(batches):
                    # drain first: with the lag-3 window this finishes head
                    # j-1's PV/l accumulation by bi==2, a full batch before
                    # its po bank partner is written again
                    if len(pendq) >= 3:
                        drain(*pendq.pop(0))
                    if bi == 2 and j > 0 and (j - 1) in po_q:
                        emit_norm(j - 1)
                    cur = []   # PV entries: (pair, e, uoff, lo_pv)
                    rsl = []   # row-sum entries: (tile, uoff|None, lo)
                    for kind, grp in batch:
                        g = grp[0] // 4
                        ng = len(grp)
                        lo_g = min(flo[i] for i in grp) & ~3
                        if kind == "n":
                            pst = psnp.tile([128, 4, 256], F32, name="pstn", tag="psn")
                            off = 256
                        else:
                            pst = psp.tile([128, 2, M], F32, name="pst", tag="ps")
                            off = 0
                        for u, i in enumerate(grp):
                            nc.tensor.matmul(
                                pst[:, u, lo_g - off: M - off],
                                kth[j][:, i * 128: (i + 1) * 128],
                                qt_t[j][:, lo_g:M],
                                start=True,
                                stop=True,
                                skip_group_check=True,
                            )
                        # ONE exp straight off PSUM for the whole group
                        # (frees the score tile), then ONE multiplicative 0/1
                        # causal mask over the partial band
                        if kind == "n":
                            e = esb.tile([128, 4, M], BF16, name="e4", tag="e4", bufs=5)
                        else:
                            e = esb.tile([128, 2, M], BF16, name="e2", tag="e2", bufs=8)
                        nc.scalar.activation(
                            e[:, :ng, lo_g:M],
                            pst[:, :ng, lo_g - off: M - off],
                            mybir.ActivationFunctionType.Exp,
                            bias=zbias[:],
                        )
                        fhi_max = max(fhi[i] for i in grp)
                        um = grp[0] % 4
                        if lo_g < fhi_max:
                            nc.vector.tensor_mul(
                                e[:, :ng, lo_g:fhi_max],
                                e[:, :ng, lo_g:fhi_max],
                                mask_t[g][:, um: um + ng, lo_g - mlo[g]: fhi_max - mlo[g]],
                            )
                        for uo in range(0, ng, 2):
                            pair = grp[uo: uo + 2]
                            lo_pv = min(flo[i] for i in pair) & ~3
                            if len(pair) == 2:
                                # pair-sum on DVE so the PE does one row-sum
                                # matmul per pair instead of per chunk
                                esum = esb.tile(
                                    [128, M], BF16, name="esum", tag="esum", bufs=14
                                )
                                nc.vector.tensor_add(
                                    esum[:, lo_pv:M],
                                    e[:, uo, lo_pv:M],
                                    e[:, uo + 1, lo_pv:M],
                                )
                                rsl.append((esum, None, lo_pv))
                            else:
                                rsl.append((e, uo, lo_pv))
                            cur.append((pair, e, uo, lo_pv))
                    pendq.append((j, cur, rsl, bi == len(batches) - 1))
                    if bi == 1:
                        ensure_kth(j + 2)
            for p in pendq:
                drain(*p)
            emit_norm(NHG - 1)

        # ---- phase C: y = O @ wo  (row-parallel partial, bf16 out) -
        with (
            nc.named_scope("phase_C"),
            tc.tile_pool(name="py", bufs=2, space="PSUM") as pyp,
            tc.tile_pool(name="ysb", bufs=2) as ysb,
        ):
            # output DMAs alternate across the two DGE queues so the 2MB of
            # y doesn't serialize on one ring at the very end; the last
            # block ships in two halves to shorten the post-matmul tail
            for mb in range(M // 128):
                ys = ysb.tile([128, D], BF16, name="ys", tag="ys")
                last = mb == M // 128 - 1
                dma_eng = nc.sync if mb % 2 == 0 else nc.scalar
                for fp in range(D // 1024):
                    py = [
                        pyp.tile([128, 512], F32, name="py", tag=f"py{2 * (fp % 2) + h}")
                        for h in range(2)
                    ]
                    for j in range(NHG):
                        for h in range(2):
                            fo = 2 * fp + h
                            nc.tensor.matmul(
                                py[h][:],
                                ot_t[j][:, mb * 128: (mb + 1) * 128],
                                wo_sb[:, j, fo * 512: (fo + 1) * 512],
                                start=(j == 0),
                                stop=(j == NHG - 1),
                                skip_group_check=True,
                            )
                    for h in range(2):
                        fo = 2 * fp + h
                        nc.scalar.copy(ys[:, fo * 512: (fo + 1) * 512], py[h][:])
                        if last:
                            dma_eng = nc.sync if fo % 2 == 0 else nc.scalar
                            dma_eng.dma_start(
                                y[mb][:, fo * 512: (fo + 1) * 512],
                                ys[:, fo * 512: (fo + 1) * 512],
                            )
                if not last:
                    dma_eng.dma_start(y[mb], ys[:])

    nc.compile()
    return nc


_cache = {}


def _get_program(flo, fhi):
    key = (tuple(flo), tuple(fhi))
    if key not in _cache:
        _cache[key] = build_program(list(flo), list(fhi))
    return _cache[key]


def _packc(a):
    """[C*128, N] row-major -> [128][C][N]: per-partition contiguous runs."""
    c = a.shape[0] // 128
    return np.ascontiguousarray(a.reshape(c, 128, a.shape[1]).transpose(1, 0, 2))


def _prep(inputs):
    x = np.asarray(inputs["x"], dtype=np.float32)
    qidx = np.asarray(inputs["query_idx"]).astype(np.int64)
    Wq = np.asarray(inputs["Wq"], dtype=np.float32)
    Wk = np.asarray(inputs["Wk"], dtype=np.float32)
    Wv = np.asarray(inputs["Wv"], dtype=np.float32)
    Wo = np.asarray(inputs["Wo"], dtype=np.float32)
    bq = np.asarray(inputs["bq"], dtype=np.float32)
    bk = np.asarray(inputs["bk"], dtype=np.float32)
    bv = np.asarray(inputs["bv"], dtype=np.float32)
    bo = np.asarray(inputs["bo"], dtype=np.float32)

    # Per-t-chunk skip bounds, union over batches.  flo[i] = first m that
    # attends into chunk i (everything below is fully masked there);
    # fhi[i] = one past the last m only partially covered by chunk i.
    # Computed positionally so they are correct even for unsorted
    # query_idx (just less effective at skipping).
    flo = [M] * NT
    fhi = [0] * NT
    for b in range(B):
        for i in range(NT):
            allowed = qidx[b] >= 128 * i          # chunk i not fully masked
            partial = qidx[b] < 128 * (i + 1)     # chunk i not fully allowed
            lo_b = int(np.argmax(allowed)) if allowed.any() else M
            hi_b = M - int(np.argmax(partial[::-1])) if partial.any() else 0
            flo[i] = min(flo[i], lo_b)
            fhi[i] = max(fhi[i], hi_b)
    mlo = [min(flo[4 * g: 4 * g + 4]) & ~3 for g in range(NT // 4)]
    mhi = [max(fhi[4 * g: 4 * g + 4]) for g in range(NT // 4)]

    in_maps = []
    tgrid = np.arange(T)[:, None]
    for core in range(8):
        b, g = divmod(core, 2)
        sl = slice(g * DG, (g + 1) * DG)
        xb = x[b]
        xT = xb.T.astype(NPBF)                                # [D, T]
        # [ts][128][ND][KTS]: per-partition 16KB contiguous runs
        xtp = np.ascontiguousarray(
            xT.reshape(ND, 128, T // KTS, KTS).transpose(2, 1, 0, 3)
        )
        xT8 = xb.T.astype(NPF8)                               # |x| << 240
        xtp8 = np.ascontiguousarray(
            xT8.reshape(ND, 128, T // KTS, KTS).transpose(2, 1, 0, 3)
        )
        wk8 = np.concatenate(
            [Wk[:, sl][:, j * 128: (j + 1) * 128] for j in FP8H], axis=1
        )
        wk8p = _packc(np.clip(wk8 * W8S, -240, 240).astype(NPF8))
        # 0/1 multiplicative causal mask
        mask = (tgrid <= qidx[b][None, :]).astype(NPBF)
        mask4 = mask.reshape(NT, 128, M)
        im = {
            "xtp": xtp,
            "xt8p": xtp8,
            "wk8p": wk8p,
            "xqp": _packc(xb[qidx[b]].T.astype(NPBF)),
            "wkp": _packc(Wk[:, sl].astype(NPBF)),
            "wvp": _packc(Wv[:, sl].astype(NPBF)),
            "wqp": _packc(Wq[:, sl].astype(NPBF)),
            "wop": _packc(Wo[sl, :].astype(NPBF)),
            "bks": np.ascontiguousarray(bk[sl].reshape(NHG, 128).T),
            "bqs": np.ascontiguousarray(
                (bq[sl] / np.sqrt(HD)).reshape(NHG, 128).T.astype(np.float32)
            ),
        }
        for g4 in range(NT // 4):
            if mlo[g4] < M and mhi[g4] > mlo[g4]:
                im[f"mask{g4}"] = np.ascontiguousarray(
                    mask4[4 * g4: 4 * g4 + 4, :, mlo[g4]: mhi[g4]].transpose(1, 0, 2)
                )
        in_maps.append(im)

    const = (bv.astype(np.float64) @ Wo.astype(np.float64) + bo).astype(np.float32)
    return flo, fhi, in_maps, const


def run(inputs, trace=False, trace_kwargs=None):
    _install_ntff_hook()
    flo, fhi, in_maps, const = _prep(inputs)
    nc = _get_program(flo, fhi)
    res = run_bass_kernel_spmd(
        nc, in_maps, list(range(8)), trace=trace, **(trace_kwargs or {})
    )
    out = np.zeros((B, M, D), dtype=np.float32)
    for b in range(B):
        out[b] = (
            res.results[2 * b]["y"].reshape(M, D).astype(np.float32)
            + res.results[2 * b + 1]["y"].reshape(M, D).astype(np.float32)
            + const
        )
    return out, res


def kernel(**inputs) -> np.ndarray:
    out, _ = run(inputs, trace=False)
    return out

